# revision 3
# baseline (speedup 1.0000x reference)
"""Causal self-attention (RMS-normed QK + RoPE + v-mix) on 8 trn2 cores.

Sharding: tensor-parallel over heads x causal-balanced query split.
  - 12 heads -> 4 groups of 3 heads; group g runs on cores (2g, 2g+1).
  - Within a pair, core parity p owns the 8 query tiles with global tile
    index == p (mod 2) (128 rows each).  Causal work per q-tile grows with
    its index, so interleaving balances the pair; every core processes the
    same uniform loop structure (SPMD), with per-core differences pushed
    into input data (weight slices, q-column gathers, rope tables, masks).
  - Each core emits a partial projection y_part = attn_g @ Wp[:,cg].T for
    its 1024 query rows; the host sums the 4 group partials per row.

Per-core kernel (all fp32):
  - q,k produced transposed [64, t] per head (heads packed 2+1 on psum
    partitions) so scores = kT.T @ qT needs no transposes; v produced in
    natural [t, 64*3] layout (x-stationary matmuls) with an interleaved
    ones column per head -> PV matmul also accumulates softmax denoms.
  - rms-norm: partition-dim sum of squares via ones-column matmul, then
    sqrt/reciprocal and a gpsimd partition-broadcast multiply.
  - rope: stacked cos/sin tables ([cos;cos;cos;cos] over partitions) so one
    multiply covers 2 heads; attention scale 1/8 folded into the q tables;
    applied in place over the raw q/k tiles.
  - softmax without max-subtraction (scores bounded by 8 after rms norm),
    causal masking via additive -1e30 masks provided as input data,
    normalization deferred past PV (flash style).
"""

import sys

sys.path.insert(0, "/opt/trn_rl_repo")

import numpy as np

import concourse.bass as bass
from concourse import mybir
from concourse.tile import TileContext
from concourse.vector_clock import ScopedClock

F32 = mybir.dt.float32
AF = mybir.ActivationFunctionType
ALU = mybir.AluOpType

T = 2048
D = 768
NH = 12
HD = 64
HPC = 3  # heads per core
C = HPC * HD  # 192 channels per group
NQ = 1024  # query rows per core
NKT = T // 128  # 16 key tiles
NDT = D // 128  # 6 contraction tiles
EPS = float(np.finfo(np.float32).eps)
MASKVAL = -1.0e30

TRACE = False
TRACE_DIR = None
_CACHED = {}


def _patch_tile_tail():
    """walrus here rejects >1 sync-wait per instruction; TileContext's tail
    drain stacks one wait per active proc.  Spread them over wait_ge's."""
    if getattr(TileContext, "_tail_patched", False):
        return

    def _drain_and_barrier(self, tick_clock, wait_clock):
        nc = self.nc
        collector = nc.sync.nop()
        wait_clock.add_sem_waits(
            collector.ins, ScopedClock({None: tick_clock.global_clock})
        )
        si = collector.ins.sync_info
        waits = list(si.on_wait) if (si and si.on_wait) else []
        if len(waits) > 1:
            by_num = {h.num: h for h in wait_clock.sems.allocated().values()}
            kept, respawn = [], []
            for w in waits:
                if kept and w.id in by_num and w.wait_mode == "sem-ge-imm":
                    respawn.append(w)
                else:
                    kept.append(w)
            si.on_wait = kept
            for w in respawn:
                nc.sync.wait_ge(by_num[w.id], w.wait_value)
        nc.sync.drain()
        nc.all_engine_barrier()
        assert self.sems is not None
        popped = nc._tile_sem_poison_stack.pop()
        assert popped is self._sem_poison
        nc.clear_and_free_semaphores(list(self.sems.allocated().values()))
        nc.all_engine_barrier()

    TileContext._drain_and_barrier = _drain_and_barrier
    TileContext._tail_patched = True


def _split_multiwait_bir(bir_json):
    """Rewrite serialized BIR so no instruction carries more than one sync
    wait (this walrus build rejects >1): extra waits move onto single-wait
    NoOps inserted just before the instruction on the same engine."""
    import json as _json

    d = _json.loads(bir_json)
    n_split = 0
    for fn in d["functions"]:
        for bb in fn["blocks"]:
            out = []
            for inst in bb["instructions"]:
                si = inst.get("sync_info") or {}
                waits = si.get("on_wait") or []
                if len(waits) > 1:
                    for wi, w in enumerate(waits[:-1]):
                        n_split += 1
                        out.append(
                            {
                                "name": f"{inst['name']}-wsplit{wi}",
                                "opcode": "EventSemaphore",
                                "engine": inst["engine"],
                                "debug": inst.get("debug", 0),
                                "ins": [],
                                "outs": [],
                                "sync_info": {"on_update": [], "on_wait": [w]},
                            }
                        )
                    si["on_wait"] = [waits[-1]]
                out.append(inst)
            bb["instructions"] = out
    enc = _json.dumps(d)
    return enc.encode() if isinstance(bir_json, bytes) else enc


def _patch_wait_split():
    import concourse.bass_utils as bu
    import concourse.bass2jax as b2j

    if getattr(bu, "_wait_split_patched", False):
        return
    orig = bu.compile_bir_kernel

    def wrapped(bir_json, tmpdir, neff_name="file.neff"):
        return orig(_split_multiwait_bir(bir_json), tmpdir, neff_name=neff_name)

    bu.compile_bir_kernel = wrapped
    b2j.compile_bir_kernel = wrapped
    bu._wait_split_patched = True


def j0_of(k):
    # first compact q-block (0..7) whose global tile can see key tile k,
    # under the uniform bound (odd-parity core's view; even cores get one
    # fully-masked diagonal block per odd k via the data mask)
    return k // 2


def build_nc():
    _patch_tile_tail()
    _patch_wait_split()
    nc = bass.Bass("TRN2")

    xt = nc.dram_tensor("xt", [D, T], F32, kind="ExternalInput")
    xq = nc.dram_tensor("xq", [D, NQ], F32, kind="ExternalInput")
    wq = nc.dram_tensor("wq", [D, C], F32, kind="ExternalInput")
    wk = nc.dram_tensor("wk", [D, C], F32, kind="ExternalInput")
    wv = nc.dram_tensor("wv", [D, C], F32, kind="ExternalInput")
    wp = nc.dram_tensor("wp", [C, D], F32, kind="ExternalInput")
    vin = nc.dram_tensor("vin", [T, C], F32, kind="ExternalInput")
    c4k = nc.dram_tensor("c4k", [128, T], F32, kind="ExternalInput")
    s4k = nc.dram_tensor("s4k", [128, T], F32, kind="ExternalInput")
    c4q = nc.dram_tensor("c4q", [128, NQ], F32, kind="ExternalInput")
    s4q = nc.dram_tensor("s4q", [128, NQ], F32, kind="ExternalInput")
    msk = nc.dram_tensor("msk", [128, NKT * 128], F32, kind="ExternalInput")
    perm = nc.dram_tensor("perm", [128, 128], F32, kind="ExternalInput")
    yp = nc.dram_tensor("yp", [NQ, D], F32, kind="ExternalOutput")

    with TileContext(nc) as tc:
        with (
            tc.tile_pool(name="const", bufs=1) as constp,
            tc.tile_pool(name="persist", bufs=1) as pers,
            tc.tile_pool(name="vpool", bufs=NKT) as vpool,
            tc.tile_pool(name="vinp", bufs=2) as vinp,
            tc.tile_pool(name="psA", bufs=2, space="PSUM") as psA,
            tc.tile_pool(name="psB", bufs=2, space="PSUM") as psB,
            tc.tile_pool(name="psbig", bufs=2, space="PSUM") as psbig,
        ):
            # ---- constants / tables ----
            ones = constp.tile([128, 64], F32, tag="ones")
            nc.vector.memset(ones[:], 1.0)
            eps_sb = constp.tile([128, 1], F32, tag="eps")
            nc.vector.memset(eps_sb[:], EPS)
            c4k_sb = constp.tile([128, T], F32, tag="c4k")
            s4k_sb = constp.tile([128, T], F32, tag="s4k")
            c4q_sb = constp.tile([128, NQ], F32, tag="c4q")
            s4q_sb = constp.tile([128, NQ], F32, tag="s4q")
            msk_sb = constp.tile([128, NKT * 128], F32, tag="msk")
            wp_sb = [
                constp.tile([64, D], F32, tag=f"wp{h}", name=f"wp{h}")
                for h in range(3)
            ]
            perm_sb = constp.tile([128, 128], F32, tag="perm")
            nc.sync.dma_start(perm_sb[:], perm[:, :])
            nc.sync.dma_start(c4k_sb[:], c4k[:, :])
            nc.sync.dma_start(s4k_sb[:], s4k[:, :])
            nc.sync.dma_start(c4q_sb[:], c4q[:, :])
            nc.sync.dma_start(s4q_sb[:], s4q[:, :])
            nc.sync.dma_start(msk_sb[:], msk[:, :])
            for h in range(3):
                nc.sync.dma_start(wp_sb[h][:], wp[64 * h : 64 * (h + 1), :])

            # ---- persistent q/k tiles (A: heads 0,1  B: head 2) ----
            # written by projection evac, rms+rope applied in place,
            # then read by the attention matmuls.
            qA = pers.tile([128, NQ], F32, tag="qA")
            qB = pers.tile([64, NQ], F32, tag="qB")
            kA = pers.tile([128, T], F32, tag="kA")
            kB = pers.tile([64, T], F32, tag="kB")

            v_sb = [
                vpool.tile([128, 3 * 65], F32, tag="v", name=f"v{t}")
                for t in range(NKT)
            ]

            with (
                tc.tile_pool(name="xtp", bufs=NDT) as xtp,
                tc.tile_pool(name="xqp", bufs=NDT) as xqp,
                tc.tile_pool(name="wsp", bufs=3 * NDT) as wsp,
                tc.tile_pool(name="scrp", bufs=2) as scrp,
                tc.tile_pool(name="scr2", bufs=4) as scr2,
                tc.tile_pool(name="rowp", bufs=2) as rowp,
                tc.tile_pool(name="bcp", bufs=2) as bcp,
            ):
                # ---- weights, hoisted (reused across chunks) ----
                wq_sb, wk_sb, wv_sb = [], [], []
                for d in range(NDT):
                    for dram, lst, nm in (
                        (wq, wq_sb, "wq"),
                        (wk, wk_sb, "wk"),
                        (wv, wv_sb, "wv"),
                    ):
                        t_ = wsp.tile([128, C], F32, tag="w", name=f"{nm}{d}")
                        nc.sync.dma_start(t_[:], dram[128 * d : 128 * (d + 1), :])
                        lst.append(t_)

                # ---- Q projection: qT[c, t], chunked over 512 columns ----
                for ch in range(NQ // 512):
                    c0 = 512 * ch
                    xq_ch = []
                    for d in range(NDT):
                        t_ = xqp.tile([128, 512], F32, tag="xq", name=f"xq{d}")
                        nc.sync.dma_start(
                            t_[:], xq[128 * d : 128 * (d + 1), c0 : c0 + 512]
                        )
                        xq_ch.append(t_)
                    for dst, m, coff in ((qA, 128, 0), (qB, 64, 128)):
                        ps = psA.tile([m, 512], F32, tag="psA", name="psq")
                        for d in range(NDT):
                            nc.tensor.matmul(
                                ps[:],
                                wq_sb[d][:, coff : coff + m],
                                xq_ch[d][:],
                                start=(d == 0),
                                stop=(d == NDT - 1),
                            )
                        nc.vector.tensor_copy(dst[:, c0 : c0 + 512], ps[:])

                # ---- K and V, merged per 512-column chunk of xt ----
                for ch in range(T // 512):
                    c0 = 512 * ch
                    xt_ch = []
                    for d in range(NDT):
                        t_ = xtp.tile([128, 512], F32, tag="xt", name=f"xt{d}")
                        nc.sync.dma_start(
                            t_[:], xt[128 * d : 128 * (d + 1), c0 : c0 + 512]
                        )
                        xt_ch.append(t_)
                    for dst, m, coff in ((kA, 128, 0), (kB, 64, 128)):
                        ps = psA.tile([m, 512], F32, tag="psA", name="psk")
                        for d in range(NDT):
                            nc.tensor.matmul(
                                ps[:],
                                wk_sb[d][:, coff : coff + m],
                                xt_ch[d][:],
                                start=(d == 0),
                                stop=(d == NDT - 1),
                            )
                        nc.vector.tensor_copy(dst[:, c0 : c0 + 512], ps[:])
                    # V for the 4 key tiles inside this chunk (natural layout)
                    for ti in range(4):
                        t = 4 * ch + ti
                        ps = psA.tile([128, 512], F32, tag="psA", name="psv")
                        for d in range(NDT):
                            nc.tensor.matmul(
                                ps[:, 0:C],
                                xt_ch[d][:, 128 * ti : 128 * (ti + 1)],
                                wv_sb[d][:],
                                start=(d == 0),
                                stop=(d == NDT - 1),
                            )
                        vi_t = vinp.tile([128, C], F32, tag="vin")
                        nc.sync.dma_start(
                            vi_t[:], vin[128 * t : 128 * (t + 1), :]
                        )
                        vt = v_sb[t]
                        dst3 = vt[:].rearrange("p (h c) -> p h c", h=3)[:, :, 0:64]
                        src3 = ps[:, 0:C].rearrange("p (h c) -> p h c", h=3)
                        vin3 = vi_t[:].rearrange("p (h c) -> p h c", h=3)
                        nc.vector.tensor_add(dst3, src3, vin3)
                        nc.vector.memset(
                            vt[:].rearrange("p (h c) -> p h c", h=3)[:, :, 64:65],
                            1.0,
                        )

                # ---- rms norm + rope, in place, chunked over 512 cols ----
                # rope is linear, so normalize last:
                #   y = (raw*cos4 + swap(raw)*sinF4) * rb
                # swap = per-64-row half-swap permutation matmul (PE moves
                # data across partitions); sinF4 = [sin;-sin;sin;-sin];
                # rb = rms reciprocal broadcast, built with K=1 matmuls.
                def rms_rope(tA, tB, cos_sb, sin_sb, n_total):
                    for ch in range(n_total // 512):
                        c0 = 512 * ch
                        for tile_, P in ((tA, 128), (tB, 64)):
                            sl = tile_[:, c0 : c0 + 512]
                            sq = scrp.tile([128, 512], F32, tag="sq", name="sq")
                            nc.scalar.square(sq[0:P, :], sl)
                            rbp = psA.tile(
                                [128, 512], F32, tag="psA", name="rbp"
                            )
                            for hh in range(P // 64):
                                po = 64 * hh
                                ssq = psA.tile(
                                    [1, 512], F32, tag="psA", name="ssq"
                                )
                                nc.tensor.matmul(
                                    ssq[:],
                                    ones[po : po + 64, 0:1],
                                    sq[po : po + 64, :],
                                    start=True,
                                    stop=True,
                                )
                                rrow = rowp.tile([1, 512], F32, tag="rrow")
                                nc.scalar.activation(
                                    rrow[:],
                                    ssq[:],
                                    AF.Sqrt,
                                    bias=eps_sb[0:1, :],
                                    scale=1.0 / HD,
                                )
                                nc.vector.reciprocal(rrow[:], rrow[:])
                                nc.tensor.matmul(
                                    rbp[po : po + 64, :],
                                    ones[0:1, 0:64],
                                    rrow[:],
                                    start=True,
                                    stop=True,
                                    tile_position=(0, po),
                                )
                            qs = psB.tile([128, 512], F32, tag="sT", name="qs")
                            nc.tensor.matmul(
                                qs[0:P, :],
                                perm_sb[0:P, 0:P],
                                sl,
                                start=True,
                                stop=True,
                            )
                            m_ = scr2.tile([128, 512], F32, tag="s2", name="m_")
                            t_ = scr2.tile([128, 512], F32, tag="s2", name="t_")
                            nc.vector.tensor_mul(
                                m_[0:P, :], sl, cos_sb[0:P, c0 : c0 + 512]
                            )
                            nc.vector.tensor_mul(
                                t_[0:P, :], qs[0:P, :], sin_sb[0:P, c0 : c0 + 512]
                            )
                            u_ = scr2.tile([128, 512], F32, tag="s2", name="u_")
                            nc.vector.tensor_add(u_[0:P, :], m_[0:P, :], t_[0:P, :])
                            nc.vector.tensor_mul(sl, u_[0:P, :], rbp[0:P, :])

                rms_rope(qA, qB, c4q_sb, s4q_sb, NQ)
                rms_rope(kA, kB, c4k_sb, s4k_sb, T)

            # ---- attention + projection ----
            with (
                tc.tile_pool(name="atp", bufs=1) as atp,
                tc.tile_pool(name="epool", bufs=3) as epool,
                tc.tile_pool(name="rowd", bufs=2) as rowd,
                tc.tile_pool(name="bcd", bufs=2) as bcd,
                tc.tile_pool(name="ypool", bufs=2) as ypool,
            ):
                at_sb = [
                    atp.tile([64, NQ], F32, tag=f"at{h}", name=f"at{h}")
                    for h in range(3)
                ]

                for h in range(3):
                    kr = kA if h < 2 else kB
                    qr = qA if h < 2 else qB
                    at = at_sb[h]
                    poff = 64 * (h % 2)
                    pv = psbig.tile([65, NQ], F32, tag="pv", name="pv")
                    for k in range(NKT):
                        q0 = 128 * j0_of(k)
                        # chunks aligned to the 512-col psum bank grid so
                        # no matmul crosses a bank boundary in pv
                        if q0 < 512:
                            pieces = [(q0, 512), (512, NQ)]
                        else:
                            pieces = [(q0, NQ)]
                        for ci, (c0, c1) in enumerate(pieces):
                            st = psB.tile([128, 512], F32, tag="sT", name="st")
                            nc.tensor.matmul(
                                st[:, 0 : c1 - c0],
                                kr[poff : poff + 64, 128 * k : 128 * (k + 1)],
                                qr[poff : poff + 64, c0:c1],
                                start=True,
                                stop=True,
                            )
                            if ci == 0:
                                nc.vector.tensor_add(
                                    st[:, 0:128],
                                    st[:, 0:128],
                                    msk_sb[:, 128 * k : 128 * (k + 1)],
                                )
                            et = epool.tile([128, 512], F32, tag="e", name="et")
                            nc.scalar.activation(
                                et[:, 0 : c1 - c0], st[:, 0 : c1 - c0], AF.Exp
                            )
                            nc.tensor.matmul(
                                pv[:, c0:c1],
                                v_sb[k][:, 65 * h : 65 * h + 65],
                                et[:, 0 : c1 - c0],
                                start=(k == 0),
                                stop=(k == NKT - 1),
                                skip_group_check=True,
                            )
                    # at = pv[0:64] * broadcast(1/pv[64]); broadcast via a
                    # K=1 ones-matmul (gpsimd partition_broadcast does not
                    # compile on this toolchain)
                    rden = rowd.tile([1, NQ], F32, tag="rden")
                    nc.vector.reciprocal(rden[:], pv[64:65, :])
                    nc.vector.tensor_copy(at[:], pv[0:64, :])
                    for c0 in range(0, NQ, 512):
                        rbn = psB.tile([64, 512], F32, tag="sT", name="rbn")
                        nc.tensor.matmul(
                            rbn[:],
                            ones[0:1, 0:64],
                            rden[:, c0 : c0 + 512],
                            start=True,
                            stop=True,
                        )
                        nc.vector.tensor_mul(
                            at[:, c0 : c0 + 512], at[:, c0 : c0 + 512], rbn[:]
                        )

                # ---- output projection: 3 K=64 slices per n-block ----
                wp_sl = [wp_sb[h][:] for h in range(3)]
                for j in range(8):
                    ps = psbig.tile([128, D], F32, tag="pv", name="psy")
                    for n0, n1 in ((0, 512), (512, D)):
                        for h in range(3):
                            nc.tensor.matmul(
                                ps[:, n0:n1],
                                at_sb[h][:, 128 * j : 128 * (j + 1)],
                                wp_sl[h][:, n0:n1],
                                start=(h == 0),
                                stop=(h == 2),
                            )
                    yt = ypool.tile([128, D], F32, tag="y")
                    nc.vector.tensor_copy(yt[:], ps[:])
                    nc.sync.dma_start(yp[128 * j : 128 * (j + 1), :], yt[:])

    return nc


def _host_prep(x, vi, Wq, Wk, Wv, Wp, lamb):
    lam = float(lamb)
    xtf = np.ascontiguousarray(x[0].T, dtype=np.float32)  # [768, 2048]

    inv_freq = (1.0 / 10000.0) ** (
        np.arange(0, HD, 2, dtype=np.float32) / HD
    )
    tpos = np.arange(T, dtype=np.float32)
    freqs = np.outer(tpos, inv_freq).astype(np.float32)  # [T, 32]
    cosT = np.cos(freqs).T.astype(np.float32)  # [32, T]
    sinT = np.sin(freqs).T.astype(np.float32)
    c4 = np.ascontiguousarray(np.vstack([cosT] * 4))  # [128, T]
    # signed sin stack: y = raw*cos4 + swap(raw)*sinF4
    #   rows 0:32  (x1): +sin * swap=x2   -> y1 = x1 c + x2 s
    #   rows 32:64 (x2): -sin * swap=x1   -> y2 = x2 c - x1 s
    s4 = np.ascontiguousarray(np.vstack([sinT, -sinT, sinT, -sinT]))
    scale = float(1.0 / np.sqrt(np.float32(HD)))
    # block-diag half-swap: qs = permf.T @ q swaps rows [0:32]<->[32:64]
    # within each 64-row head block
    permf = np.zeros((128, 128), dtype=np.float32)
    for b in range(2):
        for i in range(32):
            # column j of lhsT-perm selects: out[j] = sum_i perm[i, j]*in[i]
            permf[64 * b + 32 + i, 64 * b + i] = 1.0
            permf[64 * b + i, 64 * b + 32 + i] = 1.0

    tri = np.where(
        np.arange(128)[None, :] >= np.arange(128)[:, None], 0.0, MASKVAL
    ).astype(np.float32)  # [p=tk, c=tq]: allowed iff c >= p

    qcols_by_par = {}
    for par in (0, 1):
        jj = np.arange(8)
        qcols_by_par[par] = (
            256 * jj[:, None] + 128 * par + np.arange(128)[None, :]
        ).reshape(-1)

    in_maps = []
    for core in range(8):
        g, par = core // 2, core % 2
        cg = slice(C * g, C * (g + 1))
        qcols = qcols_by_par[par]
        mask = np.zeros((128, NKT * 128), dtype=np.float32)
        for k in range(NKT):
            gtile = 2 * j0_of(k) + par
            if gtile == k:
                mask[:, 128 * k : 128 * (k + 1)] = tri
            elif gtile < k:
                mask[:, 128 * k : 128 * (k + 1)] = MASKVAL
        in_maps.append(
            {
                "xt": xtf,
                "xq": np.ascontiguousarray(xtf[:, qcols]),
                "wq": np.ascontiguousarray(Wq[cg, :].T),
                "wk": np.ascontiguousarray(Wk[cg, :].T),
                "wv": np.ascontiguousarray((1.0 - lam) * Wv[cg, :].T),
                "wp": np.ascontiguousarray(Wp[:, cg].T),
                "vin": np.ascontiguousarray(lam * vi[0][:, cg]),
                "c4k": c4,
                "s4k": s4,
                "c4q": np.ascontiguousarray(scale * c4[:, qcols]),
                "s4q": np.ascontiguousarray(scale * s4[:, qcols]),
                "msk": mask,
                "perm": permf,
            }
        )
    return in_maps, qcols_by_par


def kernel(x, vi, Wq, Wk, Wv, Wp, lamb):
    from concourse.bass_utils import run_bass_kernel_spmd

    x = np.asarray(x, dtype=np.float32)
    vi = np.asarray(vi, dtype=np.float32)
    Wq = np.asarray(Wq, dtype=np.float32)
    Wk = np.asarray(Wk, dtype=np.float32)
    Wv = np.asarray(Wv, dtype=np.float32)
    Wp = np.asarray(Wp, dtype=np.float32)

    in_maps, qcols_by_par = _host_prep(x, vi, Wq, Wk, Wv, Wp, lamb)
    if "nc" not in _CACHED:
        _CACHED["nc"] = build_nc()
    nc = _CACHED["nc"]
    res = run_bass_kernel_spmd(
        nc, in_maps, core_ids=list(range(8)), trace=TRACE, tmpdir=TRACE_DIR
    )
    _CACHED["last_result"] = res

    y = np.zeros((T, D), dtype=np.float32)
    for core in range(8):
        y[qcols_by_par[core % 2]] += res.results[core]["yp"]
    return y[None]

